# revision 30
# baseline (speedup 1.0000x reference)
"""Trainium2 Bass kernel for the fused einsum/groupconv/bmm module.

Math (per image n, C=256, H=W=56, HW=3136, fp32):
  t1[c,e] = sum_s X[c,s] P[s,e]          (X = x[n] as C x HW, P = p1_w as HW x C)
  t3      = groupconv3x3(x[n], conv_w, groups=2)
  t4      = p4 * t1;  t5[a] = sum_b t4[a,b] p5[b]
  t6      = (t4 @ t3) / 16;  t7[s] = (sum_c t5[c] X[c,s]) / 16
  out     = t6 + t7[broadcast over c]

Device strategy (8 cores, 4 images each), all-bf16 matmul datapath
(accumulation stays fp32 in PSUM; rel-err budget 2e-2, measured ~6e-3):
  - x is PRE-PADDED and PRE-CAST TO BF16 HOST-SIDE into the flat
    (128, 3366) per-c-block layout (s' = (h+1)*58 + (w+1), one guard
    element each end): device loads are a few fat contiguous DMAs per
    image at half the fp32 traffic.  Loads are slab-split and interleaved
    with the p1 constant load so image-0 transposes and t1 start as soon
    as the first slabs land.  Image 1's prefetch is gated on a GPSIMD
    token; later images are WAR-gated naturally by the xpad rings.
  - Every matmul runs bf16 at full PE rate with FWL fast weight loads
    (LDWEIGHTS fully hidden); transposes are bf16 (1 cyc/row).
  - conv = 9 shifted matmuls per group accumulated in PSUM, chunks of 8
    padded rows (N=464), processed in chunk PAIRS with the tap loop outer
    so each stationary's LDWEIGHTS hides under two 464-wide matmuls.
  - X^T built on-chip with PE transposes into alternating halves of one
    PSUM bank; each k-chunk's two c-blocks are copied out in a single
    instruction alternating DVE/ACT; t1 matmuls for chunk k-2 interleave
    with transposes for chunk k.  Both t1 e-blocks accumulate into
    disjoint halves of ONE PSUM bank (start=True only on the very first
    matmul - start clears the whole bank's has_written bits).
  - t7 rows are computed once per chunk into a shared PSUM bank (K=1
    matmuls vs x), copied to SBUF (ACT), partition-broadcast on GPSIMD,
    and added to t6 during the PSUM->SBUF output extraction on DVE -
    no per-chunk broadcast matmuls on the PE.
  - The chunk-pair loop is software-pipelined: t6 matmuls for the
    previous pair are emitted after the current pair's conv matmuls, so
    the PE never waits on PSUM->SBUF copies.
  - Output is stored bf16 and widened to fp32 on the host.
  - 1/sqrt(C) folded into p4 host-side.
"""

import sys

sys.path.insert(0, "/opt/trn_rl_repo")

import numpy as np

N, C, H, W = 32, 256, 56, 56
HW = H * W            # 3136
PH = H + 2            # 58
PHW = PH * PH         # 3364
XLEN = PHW + 2        # +1 guard element on each end for corner tap shifts
NCORES = 8
NPER = N // NCORES    # 4 images per core
CHP = 8 * PH          # padded chunk: 8 padded rows = 464
NCHUNK = 7            # row starts 1,9,...,49 cover out rows 1..56
KP = 116              # transpose chunk (contiguous in padded space)
KT = PHW // KP        # 29
INV = 1.0 / 16.0      # 1/sqrt(C)

# x-load slabs (in flat padded coords): tiny first slab so transposes
# start the moment data lands, then fat slabs for big DMA descriptors;
# boundaries aligned to whole transpose chunks (k<=1 / k<=13 / k<=23).
XSLABS = [(0, 1 + 2 * KP), (1 + 2 * KP, 1 + 14 * KP),
          (1 + 14 * KP, 1 + 24 * KP), (1 + 24 * KP, XLEN)]
P1GROUPS = [(0, 2), (2, 16), (16, KT)]   # k-ranges per p1 load
CPAIRS = [(0, 1), (2, 3), (4, 5), (6,)]


def build_body(tc, outs, ins):
    import concourse.mybir as mybir

    nc = tc.nc
    f32 = mybir.dt.float32
    bf16 = mybir.dt.bfloat16

    x_d = ins["x"]          # (NPER, 2, 128, XLEN)  pre-padded bf16
    p1_d = ins["p1"]        # (KP, KT*C) bf16, already in SBUF layout
    wt_d = ins["wt"]        # (2, 9, 128, 128) bf16  [g, tap, i, o]
    p4_d = ins["p4s"]       # (2, 128, 256) f32  [bb, b, a]  (pre-scaled by 1/16)
    p5_d = ins["p5"]        # (2, 128) bf16  [bb, b]
    out_d = outs["out"]     # (NPER, C, HW) bf16

    with (
        tc.tile_pool(name="const", bufs=1) as constp,
        tc.tile_pool(name="xpadp", bufs=2) as xpadp,
        tc.tile_pool(name="xtp", bufs=2) as xtp,
        tc.tile_pool(name="t3p", bufs=8) as t3p,
        tc.tile_pool(name="svp", bufs=2) as svp,
        tc.tile_pool(name="t7p", bufs=4) as t7p,
        tc.tile_pool(name="outp", bufs=8) as outp,
        tc.tile_pool(name="ps_tr", bufs=1, space="PSUM") as ps_tr,
        tc.tile_pool(name="ps_acc", bufs=1, space="PSUM") as ps_acc,
        tc.tile_pool(name="ps_cv", bufs=4, space="PSUM") as ps_cv,
        tc.tile_pool(name="ps_t6", bufs=2, space="PSUM") as ps_t6,
    ):
        # ---- constants ----
        from concourse.masks import make_identity

        ident = constp.tile([128, 128], f32, name="ident")
        make_identity(nc, ident[:, :])
        identb = constp.tile([128, 128], bf16, name="identb")
        nc.vector.tensor_copy(identb[:, :], ident[:, :])

        p1_sb = constp.tile([KP, KT * C], bf16, name="p1_sb")
        wt_sb = constp.tile([128, 2 * 9 * 128], bf16, name="wt_sb")
        p4_sb = constp.tile([128, 2 * C], f32, name="p4_sb")
        p5_sb = constp.tile([128, 2], bf16, name="p5_sb")

        def load_x(n, gate):
            xps = []
            for cb in range(2):
                xp = xpadp.tile(
                    [128, XLEN], bf16, tag=f"xpad{cb}", name=f"xp{cb}"
                )
                if gate is not None:
                    # token write: forces the load DMA (WAW) to wait until
                    # `gate` exists, keeping startup bandwidth for image 0.
                    nc.gpsimd.tensor_copy(xp[:, 0:1], gate)
                for (s0, s1) in XSLABS:
                    nc.sync.dma_start(
                        out=xp[:, s0:s1], in_=x_d[n, cb, :, s0:s1]
                    )
                xps.append(xp)
            return xps

        # startup: interleave image-0 x slabs with p1 groups (both are
        # needed within the first ~15us); everything else after.
        xcur = []
        for cb in range(2):
            xp = xpadp.tile([128, XLEN], bf16, tag=f"xpad{cb}", name=f"xp{cb}")
            xcur.append(xp)

        def issue_x0_slab(si):
            s0, s1 = XSLABS[si]
            for cb in range(2):
                nc.sync.dma_start(
                    out=xcur[cb][:, s0:s1], in_=x_d[0, cb, :, s0:s1]
                )

        def issue_p1_group(gi):
            kg, k1 = P1GROUPS[gi]
            nc.sync.dma_start(
                out=p1_sb[:, kg * C : k1 * C], in_=p1_d[:, kg * C : k1 * C]
            )

        issue_x0_slab(0)
        issue_p1_group(0)
        issue_x0_slab(1)
        issue_p1_group(1)
        issue_x0_slab(2)
        issue_p1_group(2)
        issue_x0_slab(3)
        nc.sync.dma_start(
            out=wt_sb.rearrange("i (g t o) -> i g t o", g=2, t=9),
            in_=wt_d.rearrange("g t i o -> i g t o"),
        )
        nc.sync.dma_start(
            out=p4_sb.rearrange("b (bb a) -> b bb a", bb=2),
            in_=p4_d.rearrange("bb b a -> b bb a"),
        )
        nc.sync.dma_start(out=p5_sb[:, :], in_=p5_d.rearrange("bb b -> b bb"))

        for n in range(NPER):
            # ---- X^T via PE transposes + t1 (bf16), software-pipelined ----
            xt = xtp.tile([KP, KT * C], bf16, tag="xt", name="xt")
            trp = ps_tr.tile([KP, 2 * C], bf16, tag="tr", name="trp")
            t1both = ps_acc.tile([128, 2 * C], f32, tag="acc", name="t1both")

            def t1_mms(k):
                for eb in range(2):
                    nc.tensor.matmul(
                        t1both[:, eb * C : (eb + 1) * C],
                        p1_sb[:, k * C + eb * 128 : k * C + eb * 128 + 128],
                        xt[:, k * C : (k + 1) * C],
                        start=(k == 0 and eb == 0),
                        stop=(k == KT - 1),
                    )

            # transposes first, t1 after: the PE is in-order, so putting
            # every t1 matmul behind all transposes lets the transposes
            # stream while p1 (image 0) is still arriving, and by the time
            # t1 starts all xt copies are done.
            for k in range(KT):
                h0 = (k % 2) * C
                for cb in range(2):
                    nc.tensor.transpose(
                        trp[:, h0 + cb * 128 : h0 + (cb + 1) * 128],
                        xcur[cb][:, 1 + k * KP : 1 + (k + 1) * KP],
                        identb[:, :],
                    )
                if k % 2 == 0:
                    nc.vector.tensor_copy(
                        xt[:, k * C : (k + 1) * C], trp[:, h0 : h0 + C]
                    )
                else:
                    nc.scalar.copy(
                        xt[:, k * C : (k + 1) * C], trp[:, h0 : h0 + C]
                    )
            for k in range(KT):
                t1_mms(k)

            # ---- chunk helpers ----
            def t7_chunk(c):
                # t7 row for chunk c: K=1 matmuls vs x, then ACT copy to
                # SBUF and GPSIMD partition-broadcast; consumed by the DVE
                # output add one pair later.
                r0 = 1 + 8 * c
                f0 = 1 + r0 * PH
                t7ps = ps_acc.tile([1, CHP], f32, tag="acc", name="t7ps")
                for cb in range(2):
                    nc.tensor.matmul(
                        t7ps[:, :],
                        t5col[:, cb : cb + 1],
                        xcur[cb][:, f0 : f0 + CHP],
                        start=(cb == 0),
                        stop=(cb == 1),
                    )
                t7row = t7p.tile([1, CHP], f32, tag="t7row", name="t7row")
                nc.scalar.copy(t7row[:, :], t7ps[:, :])
                t7bc = t7p.tile([128, CHP], f32, tag="t7bc", name="t7bc")
                nc.gpsimd.partition_broadcast(t7bc[:, :], t7row[0:1, :])
                return t7bc

            def conv_pair(chunks, emit_t7):
                cvs = {}
                for g in range(2):
                    for c in chunks:
                        cvs[(g, c)] = ps_cv.tile(
                            [128, CHP], f32, tag="cv", name="cv"
                        )
                t7bcs = {}
                if emit_t7:
                    t7bcs[chunks[0]] = t7_chunk(chunks[0])
                for g in range(2):
                    for tap in range(9):
                        kh, kw = tap // 3, tap % 3
                        for c in chunks:
                            r0 = 1 + 8 * c
                            foff = (r0 + kh - 1) * PH + kw
                            nc.tensor.matmul(
                                cvs[(g, c)][:, :],
                                wt_sb[
                                    :,
                                    (g * 9 + tap) * 128 : (g * 9 + tap) * 128
                                    + 128,
                                ],
                                xcur[g][:, foff : foff + CHP],
                                start=(tap == 0),
                                stop=(tap == 8),
                            )
                if emit_t7 and len(chunks) > 1:
                    t7bcs[chunks[1]] = t7_chunk(chunks[1])
                out = []
                for c in chunks:
                    t3c = []
                    for g in range(2):
                        t3g = t3p.tile([128, CHP], bf16, tag="t3", name="t3g")
                        if g == 0:
                            nc.vector.tensor_copy(t3g[:, :], cvs[(g, c)][:, :])
                        else:
                            nc.scalar.copy(t3g[:, :], cvs[(g, c)][:, :])
                        t3c.append(t3g)
                    out.append((c, t3c, t7bcs.get(c)))
                return out

            def t6_chunk(c, t3c, t7bc):
                r0 = 1 + 8 * c
                tps = [
                    ps_t6.tile([128, CHP], f32, tag="t6", name="t6ps")
                    for _ in range(2)
                ]
                for bb in range(2):
                    for ab in range(2):
                        nc.tensor.matmul(
                            tps[ab][:, :],
                            t4T[:, bb * C + ab * 128 : bb * C + ab * 128 + 128],
                            t3c[bb][:, :],
                            start=(bb == 0),
                            stop=(bb == 1),
                        )
                for ab in range(2):
                    ob = outp.tile([128, 448], bf16, tag="ob", name="ob")
                    nc.vector.tensor_add(
                        ob.rearrange("p (r w) -> p r w", w=56),
                        tps[ab].rearrange("p (r w) -> p r w", w=PH)[:, :, 1:57],
                        t7bc.rearrange("p (r w) -> p r w", w=PH)[:, :, 1:57],
                    )
                    nc.sync.dma_start(
                        out=out_d[
                            n,
                            ab * 128 : (ab + 1) * 128,
                            (r0 - 1) * 56 : (r0 - 1) * 56 + 448,
                        ],
                        in_=ob[:, :],
                    )

            # ---- chunk-pair loop, software-pipelined by one pair ----
            pend = []
            for pi, chunks in enumerate(CPAIRS):
                cur = conv_pair(chunks, emit_t7=(pi > 0))
                if pi == 0:
                    # t4/t5 emitted after the first conv pair so the PE
                    # rolls straight from t1 into conv while DVE/ACT
                    # catch up; then pair 0's t7 rows (they need t5col).
                    t4T = svp.tile([128, 2 * C], bf16, tag="t4", name="t4T")
                    for eb in range(2):
                        nc.vector.tensor_mul(
                            t4T[:, eb * C : (eb + 1) * C],
                            t1both[:, eb * C : (eb + 1) * C],
                            p4_sb[:, eb * C : (eb + 1) * C],
                        )
                    t5ps = ps_acc.tile([128, 2], f32, tag="acc", name="t5ps")
                    for ab in range(2):
                        for bb in range(2):
                            nc.tensor.matmul(
                                t5ps[:, ab : ab + 1],
                                t4T[
                                    :, bb * C + ab * 128 : bb * C + ab * 128 + 128
                                ],
                                p5_sb[:, bb : bb + 1],
                                start=(bb == 0),
                                stop=(bb == 1),
                            )
                    t5col = svp.tile([128, 2], bf16, tag="t5", name="t5col")
                    nc.scalar.copy(t5col[:, :], t5ps[:, :])
                    cur = [
                        (c, t3c, t7_chunk(c)) for (c, t3c, _) in cur
                    ]
                    if n + 1 < NPER:
                        # prefetch next image.  n==0: token-gated on t4T so
                        # it doesn't steal startup bandwidth; n>=1: the
                        # xpad ring's WAR on image n-1's readers throttles.
                        xnext = load_x(
                            n + 1, t4T[:, 0:1] if n == 0 else None
                        )
                else:
                    for entry in pend:
                        t6_chunk(*entry)
                pend = cur
            for entry in pend:
                t6_chunk(*entry)

            if n + 1 < NPER:
                xcur = xnext


_CACHE = {}


def _get_nc():
    if "nc" in _CACHE:
        return _CACHE["nc"]
    import concourse.bacc as bacc
    import concourse.mybir as mybir
    import concourse.tile as tile

    f32 = mybir.dt.float32
    bf16 = mybir.dt.bfloat16
    nc = bacc.Bacc(
        "TRN2", target_bir_lowering=False, debug=False, num_devices=NCORES
    )
    ins = {
        "x": nc.dram_tensor(
            "x", (NPER, 2, 128, XLEN), bf16, kind="ExternalInput"
        ).ap(),
        "p1": nc.dram_tensor(
            "p1", (KP, KT * C), bf16, kind="ExternalInput"
        ).ap(),
        "wt": nc.dram_tensor(
            "wt", (2, 9, 128, 128), bf16, kind="ExternalInput"
        ).ap(),
        "p4s": nc.dram_tensor("p4s", (2, 128, C), f32, kind="ExternalInput").ap(),
        "p5": nc.dram_tensor("p5", (2, 128), bf16, kind="ExternalInput").ap(),
    }
    outs = {
        "out": nc.dram_tensor(
            "out", (NPER, C, HW), bf16, kind="ExternalOutput"
        ).ap(),
    }
    with tile.TileContext(nc) as tc:
        build_body(tc, outs, ins)
    nc.compile()
    _CACHE["nc"] = nc
    return nc


def host_prep(inputs):
    """Split full inputs into per-core in_maps (with host-side relayouts)."""
    import ml_dtypes

    bf = ml_dtypes.bfloat16
    x = np.asarray(inputs["x"], dtype=np.float32)
    xpad = np.zeros((N, 2, 128, XLEN), dtype=bf)
    interior = xpad[:, :, :, 60 : 60 + 56 * PH].reshape(N, 2, 128, 56, PH)
    interior[..., 0:56] = x.reshape(N, 2, 128, 56, 56).astype(bf)
    p1p = np.zeros((PH, PH, C), dtype=np.float32)
    p1p[1:57, 1:57, :] = np.asarray(inputs["p1_w"], dtype=np.float32)[..., 0]
    # relayout to the SBUF tile layout [p, k*C+e] so the device load is
    # per-partition contiguous (fat DMA descriptors)
    p1p = np.ascontiguousarray(
        p1p.reshape(KT, KP, C).transpose(1, 0, 2).reshape(KP, KT * C).astype(bf)
    )
    wt = np.ascontiguousarray(
        np.asarray(inputs["conv_w"], dtype=np.float32)
        .reshape(2, 128, 128, 9)
        .transpose(0, 3, 2, 1)
        .astype(bf)
    )
    p4s = np.ascontiguousarray(
        (np.asarray(inputs["p4_w"], dtype=np.float32)[0].T * INV).reshape(
            2, 128, C
        )
    )
    p5 = np.ascontiguousarray(
        np.asarray(inputs["p5_w"], dtype=np.float32).reshape(2, 128).astype(bf)
    )
    xs = xpad.reshape(NCORES, NPER, 2, 128, XLEN)
    return [
        {
            "x": np.ascontiguousarray(xs[i]), "p1": p1p, "wt": wt,
            "p4s": p4s, "p5": p5,
        }
        for i in range(NCORES)
    ]


def kernel(**inputs):
    from concourse.bass_utils import run_bass_kernel_spmd

    nc = _get_nc()
    in_maps = host_prep(inputs)
    res = run_bass_kernel_spmd(nc, in_maps, core_ids=list(range(NCORES)))
    out = np.concatenate(
        [
            res.results[i]["out"].astype(np.float32)
            for i in range(NCORES)
        ],
        axis=0,
    )
    return out.reshape(N, C, H, W)


# revision 32
# speedup vs baseline: 1.0036x; 1.0036x over previous
"""Trainium2 Bass kernel for the fused einsum/groupconv/bmm module.

Math (per image n, C=256, H=W=56, HW=3136, fp32):
  t1[c,e] = sum_s X[c,s] P[s,e]          (X = x[n] as C x HW, P = p1_w as HW x C)
  t3      = groupconv3x3(x[n], conv_w, groups=2)
  t4      = p4 * t1;  t5[a] = sum_b t4[a,b] p5[b]
  t6      = (t4 @ t3) / 16;  t7[s] = (sum_c t5[c] X[c,s]) / 16
  out     = t6 + t7[broadcast over c]

Device strategy (8 cores, 4 images each), all-bf16 matmul datapath
(accumulation stays fp32 in PSUM; rel-err budget 2e-2, measured ~6e-3):
  - x is PRE-PADDED and PRE-CAST TO BF16 HOST-SIDE into the flat
    (128, 3366) per-c-block layout (s' = (h+1)*58 + (w+1), one guard
    element each end): device loads are a few fat contiguous DMAs per
    image at half the fp32 traffic.  Loads are slab-split and interleaved
    with the p1 constant load so image-0 transposes and t1 start as soon
    as the first slabs land.  Image 1's prefetch is gated on a GPSIMD
    token; later images are WAR-gated naturally by the xpad rings.
  - Every matmul runs bf16 at full PE rate with FWL fast weight loads
    (LDWEIGHTS fully hidden); transposes are bf16 (1 cyc/row).
  - conv = 9 shifted matmuls per group accumulated in PSUM, chunks of 8
    padded rows (N=464), processed in chunk PAIRS with the tap loop outer
    so each stationary's LDWEIGHTS hides under two 464-wide matmuls.
  - X^T built on-chip with PE transposes into alternating halves of one
    PSUM bank; each k-chunk's two c-blocks are copied out in a single
    instruction alternating DVE/ACT; t1 matmuls for chunk k-2 interleave
    with transposes for chunk k.  Both t1 e-blocks accumulate into
    disjoint halves of ONE PSUM bank (start=True only on the very first
    matmul - start clears the whole bank's has_written bits).
  - t7 rows are computed once per chunk into a shared PSUM bank (K=1
    matmuls vs x), copied to SBUF (ACT), partition-broadcast on GPSIMD,
    and added to t6 during the PSUM->SBUF output extraction on DVE -
    no per-chunk broadcast matmuls on the PE.
  - The chunk-pair loop is software-pipelined: t6 matmuls for the
    previous pair are emitted after the current pair's conv matmuls, so
    the PE never waits on PSUM->SBUF copies.
  - Output is stored bf16 and widened to fp32 on the host.
  - 1/sqrt(C) folded into p4 host-side.
"""

import sys

sys.path.insert(0, "/opt/trn_rl_repo")

import numpy as np

N, C, H, W = 32, 256, 56, 56
HW = H * W            # 3136
PH = H + 2            # 58
PHW = PH * PH         # 3364
XLEN = PHW + 2        # +1 guard element on each end for corner tap shifts
NCORES = 8
NPER = N // NCORES    # 4 images per core
CHP = 8 * PH          # padded chunk: 8 padded rows = 464
NCHUNK = 7            # row starts 1,9,...,49 cover out rows 1..56
KP = 116              # transpose chunk (contiguous in padded space)
KT = PHW // KP        # 29
INV = 1.0 / 16.0      # 1/sqrt(C)

# x-load slabs (in flat padded coords): fat slabs for big DMA
# descriptors, boundaries aligned so transpose chunk k is inside a
# whole earlier slab (k<=13 / k<=23 / rest).
XSLABS = [(0, 1 + 14 * KP), (1 + 14 * KP, 1 + 24 * KP), (1 + 24 * KP, XLEN)]
P1GROUPS = [(0, 8), (8, 16), (16, 24), (24, KT)]   # k-ranges per p1 load
CPAIRS = [(0, 1), (2, 3), (4, 5), (6,)]


def build_body(tc, outs, ins):
    import concourse.mybir as mybir

    nc = tc.nc
    f32 = mybir.dt.float32
    bf16 = mybir.dt.bfloat16

    x_d = ins["x"]          # (NPER, 2, 128, XLEN)  pre-padded bf16
    p1_d = ins["p1"]        # (KP, KT*C) bf16, already in SBUF layout
    wt_d = ins["wt"]        # (2, 9, 128, 128) bf16  [g, tap, i, o]
    p4_d = ins["p4s"]       # (2, 128, 256) f32  [bb, b, a]  (pre-scaled by 1/16)
    p5_d = ins["p5"]        # (2, 128) bf16  [bb, b]
    out_d = outs["out"]     # (NPER, C, HW) bf16

    with (
        tc.tile_pool(name="const", bufs=1) as constp,
        tc.tile_pool(name="xpadp", bufs=2) as xpadp,
        tc.tile_pool(name="xtp", bufs=2) as xtp,
        tc.tile_pool(name="t3p", bufs=8) as t3p,
        tc.tile_pool(name="svp", bufs=2) as svp,
        tc.tile_pool(name="t7p", bufs=4) as t7p,
        tc.tile_pool(name="outp", bufs=8) as outp,
        tc.tile_pool(name="ps_tr", bufs=1, space="PSUM") as ps_tr,
        tc.tile_pool(name="ps_acc", bufs=1, space="PSUM") as ps_acc,
        tc.tile_pool(name="ps_cv", bufs=4, space="PSUM") as ps_cv,
        tc.tile_pool(name="ps_t6", bufs=2, space="PSUM") as ps_t6,
    ):
        # ---- constants ----
        from concourse.masks import make_identity

        ident = constp.tile([128, 128], f32, name="ident")
        make_identity(nc, ident[:, :])
        identb = constp.tile([128, 128], bf16, name="identb")
        nc.vector.tensor_copy(identb[:, :], ident[:, :])

        p1_sb = constp.tile([KP, KT * C], bf16, name="p1_sb")
        wt_sb = constp.tile([128, 2 * 9 * 128], bf16, name="wt_sb")
        p4_sb = constp.tile([128, 2 * C], f32, name="p4_sb")
        p5_sb = constp.tile([128, 2], bf16, name="p5_sb")

        def load_x(n, gate):
            xps = []
            for cb in range(2):
                xp = xpadp.tile(
                    [128, XLEN], bf16, tag=f"xpad{cb}", name=f"xp{cb}"
                )
                if gate is not None:
                    # token write: forces the load DMA (WAW) to wait until
                    # `gate` exists, keeping startup bandwidth for image 0.
                    nc.gpsimd.tensor_copy(xp[:, 0:1], gate)
                for (s0, s1) in XSLABS:
                    nc.sync.dma_start(
                        out=xp[:, s0:s1], in_=x_d[n, cb, :, s0:s1]
                    )
                xps.append(xp)
            return xps

        # startup: interleave image-0 x slabs with p1 groups (both are
        # needed within the first ~15us); everything else after.
        xcur = []
        for cb in range(2):
            xp = xpadp.tile([128, XLEN], bf16, tag=f"xpad{cb}", name=f"xp{cb}")
            xcur.append(xp)

        def issue_x0_slab(si):
            s0, s1 = XSLABS[si]
            for cb in range(2):
                nc.sync.dma_start(
                    out=xcur[cb][:, s0:s1], in_=x_d[0, cb, :, s0:s1]
                )

        def issue_p1_group(gi):
            kg, k1 = P1GROUPS[gi]
            nc.sync.dma_start(
                out=p1_sb[:, kg * C : k1 * C], in_=p1_d[:, kg * C : k1 * C]
            )

        issue_x0_slab(0)
        issue_p1_group(0)
        issue_x0_slab(1)
        issue_p1_group(1)
        issue_x0_slab(2)
        issue_p1_group(2)
        issue_p1_group(3)
        nc.sync.dma_start(
            out=wt_sb.rearrange("i (g t o) -> i g t o", g=2, t=9),
            in_=wt_d.rearrange("g t i o -> i g t o"),
        )
        nc.sync.dma_start(
            out=p4_sb.rearrange("b (bb a) -> b bb a", bb=2),
            in_=p4_d.rearrange("bb b a -> b bb a"),
        )
        nc.sync.dma_start(out=p5_sb[:, :], in_=p5_d.rearrange("bb b -> b bb"))

        for n in range(NPER):
            # ---- X^T via PE transposes + t1 (bf16), software-pipelined ----
            xt = xtp.tile([KP, KT * C], bf16, tag="xt", name="xt")
            trp = ps_tr.tile([KP, 2 * C], bf16, tag="tr", name="trp")
            t1both = ps_acc.tile([128, 2 * C], f32, tag="acc", name="t1both")

            def t1_mms(k):
                for eb in range(2):
                    nc.tensor.matmul(
                        t1both[:, eb * C : (eb + 1) * C],
                        p1_sb[:, k * C + eb * 128 : k * C + eb * 128 + 128],
                        xt[:, k * C : (k + 1) * C],
                        start=(k == 0 and eb == 0),
                        stop=(k == KT - 1),
                    )

            # transposes first, t1 after: the PE is in-order, so putting
            # every t1 matmul behind all transposes lets the transposes
            # stream while p1 (image 0) is still arriving, and by the time
            # t1 starts all xt copies are done.
            for k in range(KT):
                h0 = (k % 2) * C
                for cb in range(2):
                    nc.tensor.transpose(
                        trp[:, h0 + cb * 128 : h0 + (cb + 1) * 128],
                        xcur[cb][:, 1 + k * KP : 1 + (k + 1) * KP],
                        identb[:, :],
                    )
                if k % 2 == 0:
                    nc.vector.tensor_copy(
                        xt[:, k * C : (k + 1) * C], trp[:, h0 : h0 + C]
                    )
                else:
                    nc.scalar.copy(
                        xt[:, k * C : (k + 1) * C], trp[:, h0 : h0 + C]
                    )
            for k in range(KT):
                t1_mms(k)

            # ---- chunk helpers ----
            def t7_chunk(c):
                # t7 row for chunk c: K=1 matmuls vs x, then ACT copy to
                # SBUF and GPSIMD partition-broadcast; consumed by the DVE
                # output add one pair later.
                r0 = 1 + 8 * c
                f0 = 1 + r0 * PH
                t7ps = ps_acc.tile([1, CHP], f32, tag="acc", name="t7ps")
                for cb in range(2):
                    nc.tensor.matmul(
                        t7ps[:, :],
                        t5col[:, cb : cb + 1],
                        xcur[cb][:, f0 : f0 + CHP],
                        start=(cb == 0),
                        stop=(cb == 1),
                    )
                t7row = t7p.tile([1, CHP], f32, tag="t7row", name="t7row")
                nc.scalar.copy(t7row[:, :], t7ps[:, :])
                t7bc = t7p.tile([128, CHP], f32, tag="t7bc", name="t7bc")
                nc.gpsimd.partition_broadcast(t7bc[:, :], t7row[0:1, :])
                return t7bc

            def conv_pair(chunks, emit_t7):
                cvs = {}
                for g in range(2):
                    for c in chunks:
                        cvs[(g, c)] = ps_cv.tile(
                            [128, CHP], f32, tag="cv", name="cv"
                        )
                t7bcs = {}
                if emit_t7:
                    t7bcs[chunks[0]] = t7_chunk(chunks[0])
                for g in range(2):
                    for tap in range(9):
                        kh, kw = tap // 3, tap % 3
                        for c in chunks:
                            r0 = 1 + 8 * c
                            foff = (r0 + kh - 1) * PH + kw
                            nc.tensor.matmul(
                                cvs[(g, c)][:, :],
                                wt_sb[
                                    :,
                                    (g * 9 + tap) * 128 : (g * 9 + tap) * 128
                                    + 128,
                                ],
                                xcur[g][:, foff : foff + CHP],
                                start=(tap == 0),
                                stop=(tap == 8),
                            )
                if emit_t7 and len(chunks) > 1:
                    t7bcs[chunks[1]] = t7_chunk(chunks[1])
                out = []
                for c in chunks:
                    t3c = []
                    for g in range(2):
                        t3g = t3p.tile([128, CHP], bf16, tag="t3", name="t3g")
                        if g == 0:
                            nc.vector.tensor_copy(t3g[:, :], cvs[(g, c)][:, :])
                        else:
                            nc.scalar.copy(t3g[:, :], cvs[(g, c)][:, :])
                        t3c.append(t3g)
                    out.append((c, t3c, t7bcs.get(c)))
                return out

            def t6_chunk(c, t3c, t7bc):
                r0 = 1 + 8 * c
                tps = [
                    ps_t6.tile([128, CHP], f32, tag="t6", name="t6ps")
                    for _ in range(2)
                ]
                for bb in range(2):
                    for ab in range(2):
                        nc.tensor.matmul(
                            tps[ab][:, :],
                            t4T[:, bb * C + ab * 128 : bb * C + ab * 128 + 128],
                            t3c[bb][:, :],
                            start=(bb == 0),
                            stop=(bb == 1),
                        )
                for ab in range(2):
                    ob = outp.tile([128, 448], bf16, tag="ob", name="ob")
                    nc.vector.tensor_add(
                        ob.rearrange("p (r w) -> p r w", w=56),
                        tps[ab].rearrange("p (r w) -> p r w", w=PH)[:, :, 1:57],
                        t7bc.rearrange("p (r w) -> p r w", w=PH)[:, :, 1:57],
                    )
                    nc.sync.dma_start(
                        out=out_d[
                            n,
                            ab * 128 : (ab + 1) * 128,
                            (r0 - 1) * 56 : (r0 - 1) * 56 + 448,
                        ],
                        in_=ob[:, :],
                    )

            # ---- chunk-pair loop, software-pipelined by one pair ----
            pend = []
            for pi, chunks in enumerate(CPAIRS):
                cur = conv_pair(chunks, emit_t7=(pi > 0))
                if pi == 0:
                    # t4/t5 emitted after the first conv pair so the PE
                    # rolls straight from t1 into conv while DVE/ACT
                    # catch up; then pair 0's t7 rows (they need t5col).
                    t4T = svp.tile([128, 2 * C], bf16, tag="t4", name="t4T")
                    for eb in range(2):
                        nc.vector.tensor_mul(
                            t4T[:, eb * C : (eb + 1) * C],
                            t1both[:, eb * C : (eb + 1) * C],
                            p4_sb[:, eb * C : (eb + 1) * C],
                        )
                    t5ps = ps_acc.tile([128, 2], f32, tag="acc", name="t5ps")
                    for ab in range(2):
                        for bb in range(2):
                            nc.tensor.matmul(
                                t5ps[:, ab : ab + 1],
                                t4T[
                                    :, bb * C + ab * 128 : bb * C + ab * 128 + 128
                                ],
                                p5_sb[:, bb : bb + 1],
                                start=(bb == 0),
                                stop=(bb == 1),
                            )
                    t5col = svp.tile([128, 2], bf16, tag="t5", name="t5col")
                    nc.scalar.copy(t5col[:, :], t5ps[:, :])
                    cur = [
                        (c, t3c, t7_chunk(c)) for (c, t3c, _) in cur
                    ]
                    if n + 1 < NPER:
                        # prefetch next image.  n==0: token-gated on t4T so
                        # it doesn't steal startup bandwidth; n>=1: the
                        # xpad ring's WAR on image n-1's readers throttles.
                        xnext = load_x(
                            n + 1, t4T[:, 0:1] if n == 0 else None
                        )
                else:
                    for entry in pend:
                        t6_chunk(*entry)
                pend = cur
            for entry in pend:
                t6_chunk(*entry)

            if n + 1 < NPER:
                xcur = xnext


_CACHE = {}


def _get_nc():
    if "nc" in _CACHE:
        return _CACHE["nc"]
    import concourse.bacc as bacc
    import concourse.mybir as mybir
    import concourse.tile as tile

    f32 = mybir.dt.float32
    bf16 = mybir.dt.bfloat16
    nc = bacc.Bacc(
        "TRN2", target_bir_lowering=False, debug=False, num_devices=NCORES
    )
    ins = {
        "x": nc.dram_tensor(
            "x", (NPER, 2, 128, XLEN), bf16, kind="ExternalInput"
        ).ap(),
        "p1": nc.dram_tensor(
            "p1", (KP, KT * C), bf16, kind="ExternalInput"
        ).ap(),
        "wt": nc.dram_tensor(
            "wt", (2, 9, 128, 128), bf16, kind="ExternalInput"
        ).ap(),
        "p4s": nc.dram_tensor("p4s", (2, 128, C), f32, kind="ExternalInput").ap(),
        "p5": nc.dram_tensor("p5", (2, 128), bf16, kind="ExternalInput").ap(),
    }
    outs = {
        "out": nc.dram_tensor(
            "out", (NPER, C, HW), bf16, kind="ExternalOutput"
        ).ap(),
    }
    with tile.TileContext(nc) as tc:
        build_body(tc, outs, ins)
    nc.compile()
    _CACHE["nc"] = nc
    return nc


def host_prep(inputs):
    """Split full inputs into per-core in_maps (with host-side relayouts)."""
    import ml_dtypes

    bf = ml_dtypes.bfloat16
    x = np.asarray(inputs["x"], dtype=np.float32)
    xpad = np.zeros((N, 2, 128, XLEN), dtype=bf)
    interior = xpad[:, :, :, 60 : 60 + 56 * PH].reshape(N, 2, 128, 56, PH)
    interior[..., 0:56] = x.reshape(N, 2, 128, 56, 56).astype(bf)
    p1p = np.zeros((PH, PH, C), dtype=np.float32)
    p1p[1:57, 1:57, :] = np.asarray(inputs["p1_w"], dtype=np.float32)[..., 0]
    # relayout to the SBUF tile layout [p, k*C+e] so the device load is
    # per-partition contiguous (fat DMA descriptors)
    p1p = np.ascontiguousarray(
        p1p.reshape(KT, KP, C).transpose(1, 0, 2).reshape(KP, KT * C).astype(bf)
    )
    wt = np.ascontiguousarray(
        np.asarray(inputs["conv_w"], dtype=np.float32)
        .reshape(2, 128, 128, 9)
        .transpose(0, 3, 2, 1)
        .astype(bf)
    )
    p4s = np.ascontiguousarray(
        (np.asarray(inputs["p4_w"], dtype=np.float32)[0].T * INV).reshape(
            2, 128, C
        )
    )
    p5 = np.ascontiguousarray(
        np.asarray(inputs["p5_w"], dtype=np.float32).reshape(2, 128).astype(bf)
    )
    xs = xpad.reshape(NCORES, NPER, 2, 128, XLEN)
    return [
        {
            "x": np.ascontiguousarray(xs[i]), "p1": p1p, "wt": wt,
            "p4s": p4s, "p5": p5,
        }
        for i in range(NCORES)
    ]


def kernel(**inputs):
    from concourse.bass_utils import run_bass_kernel_spmd

    nc = _get_nc()
    in_maps = host_prep(inputs)
    res = run_bass_kernel_spmd(nc, in_maps, core_ids=list(range(NCORES)))
    out = np.concatenate(
        [
            res.results[i]["out"].astype(np.float32)
            for i in range(NCORES)
        ],
        axis=0,
    )
    return out.reshape(N, C, H, W)


# revision 34
# speedup vs baseline: 1.0122x; 1.0085x over previous
"""Trainium2 Bass kernel for the fused einsum/groupconv/bmm module.

Math (per image n, C=256, H=W=56, HW=3136, fp32):
  t1[c,e] = sum_s X[c,s] P[s,e]          (X = x[n] as C x HW, P = p1_w as HW x C)
  t3      = groupconv3x3(x[n], conv_w, groups=2)
  t4      = p4 * t1;  t5[a] = sum_b t4[a,b] p5[b]
  t6      = (t4 @ t3) / 16;  t7[s] = (sum_c t5[c] X[c,s]) / 16
  out     = t6 + t7[broadcast over c]

Device strategy (8 cores, 4 images each), all-bf16 matmul datapath
(accumulation stays fp32 in PSUM; rel-err budget 2e-2, measured ~6e-3):
  - x is PRE-PADDED and PRE-CAST TO BF16 HOST-SIDE into the flat
    (128, 3366) per-c-block layout (s' = (h+1)*58 + (w+1), one guard
    element each end): device loads are a few fat contiguous DMAs per
    image at half the fp32 traffic.  Loads are slab-split and interleaved
    with the p1 constant load so image-0 transposes and t1 start as soon
    as the first slabs land.  Image 1's prefetch is gated on a GPSIMD
    token; later images are WAR-gated naturally by the xpad rings.
  - Every matmul runs bf16 at full PE rate with FWL fast weight loads
    (LDWEIGHTS fully hidden); transposes are bf16 (1 cyc/row).
  - conv = 9 shifted matmuls per group accumulated in PSUM, chunks of 8
    padded rows (N=464), processed in chunk PAIRS with the tap loop outer
    so each stationary's LDWEIGHTS hides under two 464-wide matmuls.
  - X^T built on-chip with PE transposes into alternating halves of one
    PSUM bank; each k-chunk's two c-blocks are copied out in a single
    instruction alternating DVE/ACT; t1 matmuls for chunk k-2 interleave
    with transposes for chunk k.  Both t1 e-blocks accumulate into
    disjoint halves of ONE PSUM bank (start=True only on the very first
    matmul - start clears the whole bank's has_written bits).
  - t7 rows are computed once per chunk into a shared PSUM bank (K=1
    matmuls vs x), copied to SBUF (ACT), partition-broadcast on GPSIMD,
    and added to t6 during the PSUM->SBUF output extraction on DVE -
    no per-chunk broadcast matmuls on the PE.
  - The chunk-pair loop is software-pipelined: t6 matmuls for the
    previous pair are emitted after the current pair's conv matmuls, so
    the PE never waits on PSUM->SBUF copies.
  - Output is stored bf16 and widened to fp32 on the host.
  - 1/sqrt(C) folded into p4 host-side.
"""

import sys

sys.path.insert(0, "/opt/trn_rl_repo")

import numpy as np

N, C, H, W = 32, 256, 56, 56
HW = H * W            # 3136
PH = H + 2            # 58
PHW = PH * PH         # 3364
XLEN = PHW + 2        # +1 guard element on each end for corner tap shifts
NCORES = 8
NPER = N // NCORES    # 4 images per core
CHP = 8 * PH          # padded chunk: 8 padded rows = 464
NCHUNK = 7            # row starts 1,9,...,49 cover out rows 1..56
KP = 116              # transpose chunk (contiguous in padded space)
KT = PHW // KP        # 29
INV = 1.0 / 16.0      # 1/sqrt(C)

# x-load slabs (in flat padded coords): fat slabs for big DMA
# descriptors, boundaries aligned so transpose chunk k is inside a
# whole earlier slab (k<=13 / k<=23 / rest).
XSLABS = [(0, 1 + 14 * KP), (1 + 14 * KP, 1 + 24 * KP), (1 + 24 * KP, XLEN)]
P1GROUPS = [(0, 8), (8, 16), (16, 24), (24, KT)]   # k-ranges per p1 load
CPAIRS = [(0, 1), (2, 3), (4, 5), (6,)]


def build_body(tc, outs, ins):
    import concourse.mybir as mybir

    nc = tc.nc
    f32 = mybir.dt.float32
    bf16 = mybir.dt.bfloat16

    x_d = ins["x"]          # (NPER, 2, 128, XLEN)  pre-padded bf16
    p1_d = ins["p1"]        # (KP, KT*C) bf16, already in SBUF layout
    wt_d = ins["wt"]        # (2, 9, 128, 128) bf16  [g, tap, i, o]
    p4_d = ins["p4s"]       # (2, 128, 256) f32  [bb, b, a]  (pre-scaled by 1/16)
    p5_d = ins["p5"]        # (2, 128) bf16  [bb, b]
    out_d = outs["out"]     # (NPER, C, HW) bf16

    with (
        tc.tile_pool(name="const", bufs=1) as constp,
        tc.tile_pool(name="xpadp", bufs=2) as xpadp,
        tc.tile_pool(name="xtp", bufs=2) as xtp,
        tc.tile_pool(name="t3p", bufs=8) as t3p,
        tc.tile_pool(name="svp", bufs=2) as svp,
        tc.tile_pool(name="t7p", bufs=4) as t7p,
        tc.tile_pool(name="outp", bufs=8) as outp,
        tc.tile_pool(name="ps_tr", bufs=1, space="PSUM") as ps_tr,
        tc.tile_pool(name="ps_acc", bufs=1, space="PSUM") as ps_acc,
        tc.tile_pool(name="ps_cv", bufs=4, space="PSUM") as ps_cv,
        tc.tile_pool(name="ps_t6", bufs=2, space="PSUM") as ps_t6,
    ):
        # ---- constants ----
        from concourse.masks import make_identity

        ident = constp.tile([128, 128], f32, name="ident")
        make_identity(nc, ident[:, :])
        identb = constp.tile([128, 128], bf16, name="identb")
        nc.vector.tensor_copy(identb[:, :], ident[:, :])

        p1_sb = constp.tile([KP, KT * C], bf16, name="p1_sb")
        wt_sb = constp.tile([128, 2 * 9 * 128], bf16, name="wt_sb")
        p4_sb = constp.tile([128, 2 * C], f32, name="p4_sb")
        p5_sb = constp.tile([128, 2], bf16, name="p5_sb")

        def load_x(n, gate):
            xps = []
            for cb in range(2):
                xp = xpadp.tile(
                    [128, XLEN], bf16, tag=f"xpad{cb}", name=f"xp{cb}"
                )
                if gate is not None:
                    # token write: forces the load DMA (WAW) to wait until
                    # `gate` exists, keeping startup bandwidth for image 0.
                    nc.gpsimd.tensor_copy(xp[:, 0:1], gate)
                for (s0, s1) in XSLABS:
                    nc.sync.dma_start(
                        out=xp[:, s0:s1], in_=x_d[n, cb, :, s0:s1]
                    )
                xps.append(xp)
            return xps

        # startup: interleave image-0 x slabs with p1 groups (both are
        # needed within the first ~15us); everything else after.
        xcur = []
        for cb in range(2):
            xp = xpadp.tile([128, XLEN], bf16, tag=f"xpad{cb}", name=f"xp{cb}")
            xcur.append(xp)

        def issue_x0_slab(si):
            s0, s1 = XSLABS[si]
            for cb in range(2):
                nc.sync.dma_start(
                    out=xcur[cb][:, s0:s1], in_=x_d[0, cb, :, s0:s1]
                )

        def issue_p1_group(gi):
            kg, k1 = P1GROUPS[gi]
            nc.sync.dma_start(
                out=p1_sb[:, kg * C : k1 * C], in_=p1_d[:, kg * C : k1 * C]
            )

        issue_x0_slab(0)
        issue_p1_group(0)
        issue_x0_slab(1)
        issue_p1_group(1)
        issue_x0_slab(2)
        issue_p1_group(2)
        issue_p1_group(3)
        nc.sync.dma_start(
            out=wt_sb.rearrange("i (g t o) -> i g t o", g=2, t=9),
            in_=wt_d.rearrange("g t i o -> i g t o"),
        )
        nc.sync.dma_start(
            out=p4_sb.rearrange("b (bb a) -> b bb a", bb=2),
            in_=p4_d.rearrange("bb b a -> b bb a"),
        )
        nc.sync.dma_start(out=p5_sb[:, :], in_=p5_d.rearrange("bb b -> b bb"))

        for n in range(NPER):
            # ---- X^T via PE transposes + t1 (bf16), software-pipelined ----
            xt = xtp.tile([KP, KT * C], bf16, tag="xt", name="xt")
            trp = ps_tr.tile([KP, 2 * C], bf16, tag="tr", name="trp")
            t1both = ps_acc.tile([128, 2 * C], f32, tag="acc", name="t1both")

            def t1_mms(k):
                for eb in range(2):
                    nc.tensor.matmul(
                        t1both[:, eb * C : (eb + 1) * C],
                        p1_sb[:, k * C + eb * 128 : k * C + eb * 128 + 128],
                        xt[:, k * C : (k + 1) * C],
                        start=(k == 0 and eb == 0),
                        stop=(k == KT - 1),
                    )

            # transposes first, t1 after: the PE is in-order, so putting
            # every t1 matmul behind all transposes lets the transposes
            # stream while p1 (image 0) is still arriving, and by the time
            # t1 starts all xt copies are done.
            for k in range(KT):
                h0 = (k % 2) * C
                for cb in range(2):
                    nc.tensor.transpose(
                        trp[:, h0 + cb * 128 : h0 + (cb + 1) * 128],
                        xcur[cb][:, 1 + k * KP : 1 + (k + 1) * KP],
                        identb[:, :],
                    )
                if k % 2 == 0:
                    nc.vector.tensor_copy(
                        xt[:, k * C : (k + 1) * C], trp[:, h0 : h0 + C]
                    )
                else:
                    nc.scalar.copy(
                        xt[:, k * C : (k + 1) * C], trp[:, h0 : h0 + C]
                    )

            # ---- chunk helpers ----
            def t7_chunk(c):
                # t7 row for chunk c: K=1 matmuls vs x, then ACT copy to
                # SBUF and GPSIMD partition-broadcast; consumed by the DVE
                # output add one pair later.
                r0 = 1 + 8 * c
                f0 = 1 + r0 * PH
                t7ps = ps_acc.tile([1, CHP], f32, tag="acc", name="t7ps")
                for cb in range(2):
                    nc.tensor.matmul(
                        t7ps[:, :],
                        t5col[:, cb : cb + 1],
                        xcur[cb][:, f0 : f0 + CHP],
                        start=(cb == 0),
                        stop=(cb == 1),
                    )
                t7row = t7p.tile([1, CHP], f32, tag="t7row", name="t7row")
                nc.scalar.copy(t7row[:, :], t7ps[:, :])
                t7bc = t7p.tile([128, CHP], f32, tag="t7bc", name="t7bc")
                nc.gpsimd.partition_broadcast(t7bc[:, :], t7row[0:1, :])
                return t7bc

            def conv_pair(chunks, emit_t7):
                cvs = {}
                for g in range(2):
                    for c in chunks:
                        cvs[(g, c)] = ps_cv.tile(
                            [128, CHP], f32, tag="cv", name="cv"
                        )
                t7bcs = {}
                if emit_t7:
                    t7bcs[chunks[0]] = t7_chunk(chunks[0])
                for g in range(2):
                    for tap in range(9):
                        kh, kw = tap // 3, tap % 3
                        for c in chunks:
                            r0 = 1 + 8 * c
                            foff = (r0 + kh - 1) * PH + kw
                            nc.tensor.matmul(
                                cvs[(g, c)][:, :],
                                wt_sb[
                                    :,
                                    (g * 9 + tap) * 128 : (g * 9 + tap) * 128
                                    + 128,
                                ],
                                xcur[g][:, foff : foff + CHP],
                                start=(tap == 0),
                                stop=(tap == 8),
                            )
                if emit_t7 and len(chunks) > 1:
                    t7bcs[chunks[1]] = t7_chunk(chunks[1])
                out = []
                for c in chunks:
                    t3c = []
                    for g in range(2):
                        t3g = t3p.tile([128, CHP], bf16, tag="t3", name="t3g")
                        if g == 0:
                            nc.vector.tensor_copy(t3g[:, :], cvs[(g, c)][:, :])
                        else:
                            nc.scalar.copy(t3g[:, :], cvs[(g, c)][:, :])
                        t3c.append(t3g)
                    out.append((c, t3c, t7bcs.get(c)))
                return out

            def t6_chunk(c, t3c, t7bc):
                r0 = 1 + 8 * c
                tps = [
                    ps_t6.tile([128, CHP], f32, tag="t6", name="t6ps")
                    for _ in range(2)
                ]
                for bb in range(2):
                    for ab in range(2):
                        nc.tensor.matmul(
                            tps[ab][:, :],
                            t4T[:, bb * C + ab * 128 : bb * C + ab * 128 + 128],
                            t3c[bb][:, :],
                            start=(bb == 0),
                            stop=(bb == 1),
                        )
                for ab in range(2):
                    ob = outp.tile([128, 448], bf16, tag="ob", name="ob")
                    nc.vector.tensor_add(
                        ob.rearrange("p (r w) -> p r w", w=56),
                        tps[ab].rearrange("p (r w) -> p r w", w=PH)[:, :, 1:57],
                        t7bc.rearrange("p (r w) -> p r w", w=PH)[:, :, 1:57],
                    )
                    nc.sync.dma_start(
                        out=out_d[
                            n,
                            ab * 128 : (ab + 1) * 128,
                            (r0 - 1) * 56 : (r0 - 1) * 56 + 448,
                        ],
                        in_=ob[:, :],
                    )

            # ---- chunk-pair loop, software-pipelined by one pair ----
            # conv pair 0 is emitted BEFORE the t1 matmuls: it depends
            # only on x, so (image 0) the PE chews through it while p1 is
            # still arriving instead of stalling at t1.
            pend = []
            for pi, chunks in enumerate(CPAIRS):
                if pi == 0:
                    cur = conv_pair(chunks, emit_t7=False)
                    for k in range(KT):
                        t1_mms(k)
                else:
                    cur = conv_pair(chunks, emit_t7=True)
                if pi == 0:
                    # t4/t5 emitted after the first conv pair so the PE
                    # rolls straight from t1 into conv while DVE/ACT
                    # catch up; then pair 0's t7 rows (they need t5col).
                    t4T = svp.tile([128, 2 * C], bf16, tag="t4", name="t4T")
                    for eb in range(2):
                        nc.vector.tensor_mul(
                            t4T[:, eb * C : (eb + 1) * C],
                            t1both[:, eb * C : (eb + 1) * C],
                            p4_sb[:, eb * C : (eb + 1) * C],
                        )
                    t5ps = ps_acc.tile([128, 2], f32, tag="acc", name="t5ps")
                    for ab in range(2):
                        for bb in range(2):
                            nc.tensor.matmul(
                                t5ps[:, ab : ab + 1],
                                t4T[
                                    :, bb * C + ab * 128 : bb * C + ab * 128 + 128
                                ],
                                p5_sb[:, bb : bb + 1],
                                start=(bb == 0),
                                stop=(bb == 1),
                            )
                    t5col = svp.tile([128, 2], bf16, tag="t5", name="t5col")
                    nc.scalar.copy(t5col[:, :], t5ps[:, :])
                    cur = [
                        (c, t3c, t7_chunk(c)) for (c, t3c, _) in cur
                    ]
                    if n + 1 < NPER:
                        # prefetch next image.  n==0: token-gated on t4T so
                        # it doesn't steal startup bandwidth; n>=1: the
                        # xpad ring's WAR on image n-1's readers throttles.
                        xnext = load_x(
                            n + 1, t4T[:, 0:1] if n == 0 else None
                        )
                else:
                    for entry in pend:
                        t6_chunk(*entry)
                pend = cur
            for entry in pend:
                t6_chunk(*entry)

            if n + 1 < NPER:
                xcur = xnext


_CACHE = {}


def _get_nc():
    if "nc" in _CACHE:
        return _CACHE["nc"]
    import concourse.bacc as bacc
    import concourse.mybir as mybir
    import concourse.tile as tile

    f32 = mybir.dt.float32
    bf16 = mybir.dt.bfloat16
    nc = bacc.Bacc(
        "TRN2", target_bir_lowering=False, debug=False, num_devices=NCORES
    )
    ins = {
        "x": nc.dram_tensor(
            "x", (NPER, 2, 128, XLEN), bf16, kind="ExternalInput"
        ).ap(),
        "p1": nc.dram_tensor(
            "p1", (KP, KT * C), bf16, kind="ExternalInput"
        ).ap(),
        "wt": nc.dram_tensor(
            "wt", (2, 9, 128, 128), bf16, kind="ExternalInput"
        ).ap(),
        "p4s": nc.dram_tensor("p4s", (2, 128, C), f32, kind="ExternalInput").ap(),
        "p5": nc.dram_tensor("p5", (2, 128), bf16, kind="ExternalInput").ap(),
    }
    outs = {
        "out": nc.dram_tensor(
            "out", (NPER, C, HW), bf16, kind="ExternalOutput"
        ).ap(),
    }
    with tile.TileContext(nc) as tc:
        build_body(tc, outs, ins)
    nc.compile()
    _CACHE["nc"] = nc
    return nc


def host_prep(inputs):
    """Split full inputs into per-core in_maps (with host-side relayouts)."""
    import ml_dtypes

    bf = ml_dtypes.bfloat16
    x = np.asarray(inputs["x"], dtype=np.float32)
    xpad = np.zeros((N, 2, 128, XLEN), dtype=bf)
    interior = xpad[:, :, :, 60 : 60 + 56 * PH].reshape(N, 2, 128, 56, PH)
    interior[..., 0:56] = x.reshape(N, 2, 128, 56, 56).astype(bf)
    p1p = np.zeros((PH, PH, C), dtype=np.float32)
    p1p[1:57, 1:57, :] = np.asarray(inputs["p1_w"], dtype=np.float32)[..., 0]
    # relayout to the SBUF tile layout [p, k*C+e] so the device load is
    # per-partition contiguous (fat DMA descriptors)
    p1p = np.ascontiguousarray(
        p1p.reshape(KT, KP, C).transpose(1, 0, 2).reshape(KP, KT * C).astype(bf)
    )
    wt = np.ascontiguousarray(
        np.asarray(inputs["conv_w"], dtype=np.float32)
        .reshape(2, 128, 128, 9)
        .transpose(0, 3, 2, 1)
        .astype(bf)
    )
    p4s = np.ascontiguousarray(
        (np.asarray(inputs["p4_w"], dtype=np.float32)[0].T * INV).reshape(
            2, 128, C
        )
    )
    p5 = np.ascontiguousarray(
        np.asarray(inputs["p5_w"], dtype=np.float32).reshape(2, 128).astype(bf)
    )
    xs = xpad.reshape(NCORES, NPER, 2, 128, XLEN)
    return [
        {
            "x": np.ascontiguousarray(xs[i]), "p1": p1p, "wt": wt,
            "p4s": p4s, "p5": p5,
        }
        for i in range(NCORES)
    ]


def kernel(**inputs):
    from concourse.bass_utils import run_bass_kernel_spmd

    nc = _get_nc()
    in_maps = host_prep(inputs)
    res = run_bass_kernel_spmd(nc, in_maps, core_ids=list(range(NCORES)))
    out = np.concatenate(
        [
            res.results[i]["out"].astype(np.float32)
            for i in range(NCORES)
        ],
        axis=0,
    )
    return out.reshape(N, C, H, W)
